# revision 9
# baseline (speedup 1.0000x reference)
"""GCN (3-layer GCNConv + linear head) on 8 Trainium2 NeuronCores.

Strategy (1D node partition, per sharding hint):
 - Host: add self-loops, compute deg/dinv, sort nodes by degree, deal
   128-node blocks round-robin to 8 cores (load balance), build padded-CSR
   gather index lists per (block, src-chunk) with int16 chunk-local indices.
 - Device (SPMD, same graph on 8 cores): per layer, each core computes its
   shard of the scaled feature table T = dinv * (X @ W), AllGathers the
   4 chunk-tables, then per dst-block does dma_gather (padded CSR) +
   TensorE matmul-reduction against diag(dinv) (fuses transpose + the
   dst-side normalization), bias/relu on ScalarE.
 - norm separability: norm_e = dinv[src]*dinv[dst], so messages are just
   rows of the pre-scaled table; no per-edge arithmetic anywhere.
"""
import sys

sys.path.insert(0, "/opt/trn_rl_repo")

import numpy as np

N_CORES = 8
F_IN, F_HID, N_CLS = 128, 64, 16
P = 128
N_QUARTERS = 4


class Cfg:
    def __init__(self, n_nodes=100000, blocks_per_core=104):
        self.n_nodes = n_nodes
        self.bpc = blocks_per_core
        assert blocks_per_core % N_QUARTERS == 0
        self.per_core = self.bpc * P
        self.bpq = self.bpc // N_QUARTERS
        self.qrows = self.bpq * P
        self.chunk_rows = self.qrows * N_CORES
        self.n_pad = self.per_core * N_CORES
        self.n_blocks = self.bpc * N_CORES
        assert self.n_pad >= n_nodes + P * N_QUARTERS, "need dummy blocks"
        assert self.chunk_rows <= 32767, "int16 index range"


FULL = Cfg()


def _preprocess(cfg, x, edge_index):
    """Returns per-core device inputs + the (core, lb, p) placement of each node."""
    n = cfg.n_nodes
    src = edge_index[0].astype(np.int64)
    dst = edge_index[1].astype(np.int64)
    loops = np.arange(n, dtype=np.int64)
    src = np.concatenate([src, loops])
    dst = np.concatenate([dst, loops])

    deg = np.bincount(dst, minlength=n).astype(np.float64)
    dinv = np.where(deg > 0, 1.0 / np.sqrt(deg), 0.0).astype(np.float32)

    order = np.argsort(-deg, kind="stable")
    rank = np.empty(n, dtype=np.int64)
    rank[order] = np.arange(n)

    # rank r -> block b, pos p; block b -> core b%8, seq b//8;
    # core-local order spreads quarters: lb = (seq%4)*bpq + seq//4
    r = rank
    b = r // P
    p_of = (r % P).astype(np.int64)
    core_of = (b % N_CORES).astype(np.int64)
    seq = b // N_CORES
    q_of = (seq % N_QUARTERS).astype(np.int64)
    lq_of = (seq // N_QUARTERS).astype(np.int64)
    lb_of = q_of * cfg.bpq + lq_of
    chunkrow_of = core_of * cfg.qrows + lq_of * P + p_of

    # a guaranteed all-dummy row per chunk (for padding indices)
    zero_row_of_chunk = np.zeros(N_QUARTERS, dtype=np.int64)
    found = [False] * N_QUARTERS
    for bb in range(n // P + 1, cfg.n_blocks):
        cc = bb % N_CORES
        ss = bb // N_CORES
        qq = ss % N_QUARTERS
        lql = ss // N_QUARTERS
        if not found[qq]:
            zero_row_of_chunk[qq] = cc * cfg.qrows + lql * P
            found[qq] = True
    assert all(found)

    d_core = core_of[dst]
    d_lb = lb_of[dst]
    d_p = p_of[dst]
    s_q = q_of[src]
    s_row = chunkrow_of[src]

    key = ((d_core * cfg.bpc + d_lb) * N_QUARTERS + s_q) * P + d_p
    sort_idx = np.argsort(key, kind="stable")
    key_s = key[sort_idx]
    s_row_s = s_row[sort_idx]
    n_groups = N_CORES * cfg.bpc * N_QUARTERS * P
    counts = np.bincount(key_s, minlength=n_groups)
    group_starts = np.concatenate([[0], np.cumsum(counts)[:-1]])
    k_in_group = np.arange(len(key_s)) - group_starts[key_s]

    counts_4d = counts.reshape(N_CORES, cfg.bpc, N_QUARTERS, P)
    K_call = counts_4d.max(axis=(0, 3))            # [bpc, 4], same for all cores
    Kmax = int(K_call.max())

    call_cols = 8 * K_call
    col_off = np.zeros((cfg.bpc, N_QUARTERS), dtype=np.int64)
    acc = 0
    for lb in range(cfg.bpc):
        for c in range(N_QUARTERS):
            col_off[lb, c] = acc
            acc += call_cols[lb, c]
    IDXW = int(acc)

    idx16 = np.zeros((N_CORES, 16, IDXW), dtype=np.int16)
    for lb in range(cfg.bpc):
        for c in range(N_QUARTERS):
            if K_call[lb, c]:
                idx16[:, :, col_off[lb, c]:col_off[lb, c] + call_cols[lb, c]] = \
                    np.int16(zero_row_of_chunk[c])

    e_core = d_core[sort_idx]
    e_lb = d_lb[sort_idx]
    e_c = s_q[sort_idx]
    e_p = d_p[sort_idx]
    e_i = k_in_group * P + e_p
    e_col = col_off[e_lb, e_c] + e_i // 16
    idx16[e_core, e_i % 16, e_col] = s_row_s.astype(np.int16)

    idx_full = np.broadcast_to(idx16[:, None, :, :], (N_CORES, 8, 16, IDXW)) \
        .reshape(N_CORES, 128, IDXW).copy()

    xT = np.zeros((N_CORES, F_IN, cfg.per_core), dtype=np.float32)
    dinv_pp = np.zeros((N_CORES, P, cfg.bpc), dtype=np.float32)
    local_col = lb_of * P + p_of
    xT[core_of, :, local_col] = x.astype(np.float32)
    dinv_pp[core_of, p_of, lb_of] = dinv

    meta = dict(K_call=K_call, col_off=col_off, IDXW=IDXW, Kmax=Kmax,
                core_of=core_of, local_of=local_col)
    return xT, dinv_pp, idx_full, meta


def _build_bass(cfg, meta):
    import concourse.bacc as bacc
    import concourse.mybir as mybir
    from concourse import tile

    K_call = meta["K_call"]
    col_off = meta["col_off"]
    IDXW = meta["IDXW"]
    Kmax = meta["Kmax"]
    f32 = mybir.dt.float32
    AF = mybir.ActivationFunctionType

    nc = bacc.Bacc("TRN2")
    xT_in = nc.dram_tensor("xT", [F_IN, cfg.per_core], f32, kind="ExternalInput")
    idx_in = nc.dram_tensor("idx", [P, IDXW], mybir.dt.int16, kind="ExternalInput")
    dinv_in = nc.dram_tensor("dinv_pp", [P, cfg.bpc], f32, kind="ExternalInput")
    w0_in = nc.dram_tensor("w0", [F_IN, F_HID], f32, kind="ExternalInput")
    w1_in = nc.dram_tensor("w1", [F_HID, F_HID], f32, kind="ExternalInput")
    w2_in = nc.dram_tensor("w2", [F_HID, F_HID], f32, kind="ExternalInput")
    wc_in = nc.dram_tensor("wc", [F_HID, N_CLS], f32, kind="ExternalInput")
    b0_in = nc.dram_tensor("b0", [F_HID, 1], f32, kind="ExternalInput")
    b1_in = nc.dram_tensor("b1", [F_HID, 1], f32, kind="ExternalInput")
    b2_in = nc.dram_tensor("b2", [F_HID, 1], f32, kind="ExternalInput")
    bc_in = nc.dram_tensor("bc", [N_CLS, 1], f32, kind="ExternalInput")
    ident_in = nc.dram_tensor("ident", [P, P], f32, kind="ExternalInput")
    out_ext = nc.dram_tensor("out", [cfg.per_core, N_CLS], f32,
                             kind="ExternalOutput")

    rg = [list(range(N_CORES))]

    with tile.TileContext(nc) as tc:
        with (
            tc.tile_pool(name="const", bufs=1) as cpool,
            tc.tile_pool(name="idxp", bufs=1) as ipool,
            tc.tile_pool(name="gb", bufs=4) as gpool,
            tc.tile_pool(name="sm", bufs=4) as spool,
            tc.tile_pool(name="rows", bufs=4) as rpool,
            tc.tile_pool(name="xin", bufs=4) as xpool,
            tc.tile_pool(name="ps", bufs=2, space="PSUM") as pspool,
            tc.tile_pool(name="ps2", bufs=1, space="PSUM") as ps2pool,
            tc.tile_pool(name="dram", bufs=1, space="DRAM") as dpool,
        ):
            w0_t = cpool.tile([F_IN, F_HID], f32)
            w1_t = cpool.tile([F_HID, F_HID], f32)
            w2_t = cpool.tile([F_HID, F_HID], f32)
            wc_t = cpool.tile([F_HID, N_CLS], f32)
            b0_t = cpool.tile([F_HID, 1], f32)
            b1_t = cpool.tile([F_HID, 1], f32)
            b2_t = cpool.tile([F_HID, 1], f32)
            bc_t = cpool.tile([N_CLS, 1], f32)
            ident_t = cpool.tile([P, P], f32)
            dinv_t = cpool.tile([P, cfg.bpc], f32)
            idx_t = ipool.tile([P, IDXW], mybir.dt.int16)
            for t, src_ in ((w0_t, w0_in), (w1_t, w1_in), (w2_t, w2_in),
                            (wc_t, wc_in), (b0_t, b0_in), (b1_t, b1_in),
                            (b2_t, b2_in), (bc_t, bc_in), (ident_t, ident_in),
                            (dinv_t, dinv_in), (idx_t, idx_in)):
                nc.sync.dma_start(out=t[:], in_=src_[:])

            shards = [dpool.tile([cfg.per_core, F_HID], f32, tag=f"shard{i}",
                                 name=f"shard{i}") for i in range(3)]
            chunk_tabs = [[dpool.tile([cfg.chunk_rows, F_HID], f32,
                                      tag=f"ct{i}_{q}", name=f"ct{i}_{q}",
                                      addr_space="Shared")
                           for q in range(N_QUARTERS)] for i in range(3)]

            def build_table_rows(lb, psum3, layer):
                """psum3 [64,128] = (X@W).T block -> scaled rows -> shard."""
                tt = spool.tile([F_HID, P], f32, tag="tt", name="tt")
                nc.scalar.activation(out=tt[:], in_=psum3[:], func=AF.Copy)
                psum4 = ps2pool.tile([P, F_HID], f32, tag="p4", name="p4")
                nc.tensor.transpose(out=psum4[:], in_=tt[:],
                                    identity=ident_t[:F_HID, :F_HID])
                row = rpool.tile([P, F_HID], f32, tag="row", name="row")
                nc.scalar.activation(out=row[:], in_=psum4[:], func=AF.Copy,
                                     scale=dinv_t[:, lb:lb + 1])
                nc.sync.dma_start(out=shards[layer][lb * P:(lb + 1) * P, :],
                                  in_=row[:])

            # ---- Phase A: table0 = dinv * (x @ W0) ----
            for lb in range(cfg.bpc):
                xt = xpool.tile([F_IN, P], f32, tag="xt", name="xt")
                nc.sync.dma_start(out=xt[:], in_=xT_in[:, lb * P:(lb + 1) * P])
                psum3 = pspool.tile([F_HID, P], f32, tag="p3", name="p3")
                nc.tensor.matmul(out=psum3[:], lhsT=w0_t[:], rhs=xt[:],
                                 start=True, stop=True)
                build_table_rows(lb, psum3, 0)

            def allgather(layer):
                for q in range(N_QUARTERS):
                    nc.gpsimd.collective_compute(
                        "AllGather", mybir.AluOpType.bypass,
                        replica_groups=rg,
                        ins=[shards[layer][q * cfg.qrows:(q + 1) * cfg.qrows, :].opt()],
                        outs=[chunk_tabs[layer][q][:].opt()],
                    )

            allgather(0)

            def gather_reduce(layer, lb):
                """psumS [64,128] = segsum.T scaled by dinv[dst], or None."""
                if K_call[lb].sum() == 0:
                    return None
                D_t = spool.tile([P, P], f32, tag="D", name="D")
                nc.vector.tensor_scalar_mul(D_t[:], ident_t[:], dinv_t[:, lb:lb + 1])
                psumS = pspool.tile([F_HID, P], f32, tag="pS", name="pS")
                first = True
                nmm = int(K_call[lb].sum())
                done = 0
                for c in range(N_QUARTERS):
                    K = int(K_call[lb, c])
                    if K == 0:
                        continue
                    g = gpool.tile([P, Kmax * F_HID], f32, tag="g", name="g")
                    g3 = g[:].rearrange("p (k f) -> p k f", f=F_HID)
                    # dma_gather is limited to 1024 indices (8 slots) per call
                    for k0 in range(0, K, 8):
                        kk = min(8, K - k0)
                        nc.gpsimd.dma_gather(
                            g3[:, k0:k0 + kk, :],
                            chunk_tabs[layer][c][:],
                            idx_t[:, col_off[lb, c] + 8 * k0:
                                  col_off[lb, c] + 8 * (k0 + kk)],
                            kk * P, kk * P, F_HID,
                        )
                    for k in range(K):
                        done += 1
                        nc.tensor.matmul(
                            out=psumS[:],
                            lhsT=g[:, k * F_HID:(k + 1) * F_HID],
                            rhs=D_t[:],
                            start=first, stop=(done == nmm),
                        )
                        first = False
                return psumS

            # ---- Phases B,C: conv layers 0,1 -> table1, table2 ----
            for layer, (w_next, b_l) in enumerate(((w1_t, b0_t), (w2_t, b1_t))):
                for lb in range(cfg.bpc):
                    psumS = gather_reduce(layer, lb)
                    if psumS is None:
                        row = rpool.tile([P, F_HID], f32, tag="row", name="row")
                        nc.vector.memset(row[:], 0.0)
                        nc.sync.dma_start(
                            out=shards[layer + 1][lb * P:(lb + 1) * P, :],
                            in_=row[:])
                        continue
                    a_sb = spool.tile([F_HID, P], f32, tag="a", name="a")
                    nc.scalar.activation(out=a_sb[:], in_=psumS[:], func=AF.Relu,
                                         bias=b_l[:])
                    psum3 = pspool.tile([F_HID, P], f32, tag="p3", name="p3")
                    nc.tensor.matmul(out=psum3[:], lhsT=w_next[:], rhs=a_sb[:],
                                     start=True, stop=True)
                    build_table_rows(lb, psum3, layer + 1)
                allgather(layer + 1)

            # ---- Phase D: conv layer 2 + classifier ----
            for lb in range(cfg.bpc):
                psumS = gather_reduce(2, lb)
                lrow = rpool.tile([P, N_CLS], f32, tag="lrow", name="lrow")
                if psumS is None:
                    nc.vector.memset(lrow[:], 0.0)
                else:
                    o_sb = spool.tile([F_HID, P], f32, tag="a", name="a")
                    nc.vector.tensor_scalar_add(o_sb[:], psumS[:], b2_t[:])
                    psumL = ps2pool.tile([N_CLS, P], f32, tag="pL", name="pL")
                    nc.tensor.matmul(out=psumL[:], lhsT=wc_t[:], rhs=o_sb[:],
                                     start=True, stop=True)
                    lt = spool.tile([N_CLS, P], f32, tag="lt", name="lt")
                    nc.vector.tensor_scalar_add(lt[:], psumL[:], bc_t[:])
                    psumT = ps2pool.tile([P, N_CLS], f32, tag="pT", name="pT")
                    nc.tensor.transpose(out=psumT[:], in_=lt[:],
                                        identity=ident_t[:N_CLS, :N_CLS])
                    nc.scalar.activation(out=lrow[:], in_=psumT[:], func=AF.Copy)
                nc.sync.dma_start(out=out_ext[lb * P:(lb + 1) * P, :], in_=lrow[:])

    nc.finalize()
    return nc


LAST_NC = None
LAST_RESULTS = None


def make_in_maps(xT, dinv_pp, idx_full, W0, b0, W1, b1, W2, b2, Wc, bc):
    ident = np.eye(P, dtype=np.float32)
    in_maps = []
    for c in range(N_CORES):
        in_maps.append({
            "xT": np.ascontiguousarray(xT[c]),
            "idx": np.ascontiguousarray(idx_full[c]),
            "dinv_pp": np.ascontiguousarray(dinv_pp[c]),
            "w0": np.asarray(W0, np.float32),
            "w1": np.asarray(W1, np.float32),
            "w2": np.asarray(W2, np.float32),
            "wc": np.asarray(Wc, np.float32),
            "b0": np.asarray(b0, np.float32).reshape(F_HID, 1),
            "b1": np.asarray(b1, np.float32).reshape(F_HID, 1),
            "b2": np.asarray(b2, np.float32).reshape(F_HID, 1),
            "bc": np.asarray(bc, np.float32).reshape(N_CLS, 1),
            "ident": ident,
        })
    return in_maps


def kernel(x, edge_index, W0, b0, W1, b1, W2, b2, Wc, bc):
    from concourse.bass_utils import run_bass_kernel_spmd

    cfg = FULL
    x = np.asarray(x)
    edge_index = np.asarray(edge_index)
    xT, dinv_pp, idx_full, meta = _preprocess(cfg, x, edge_index)
    nc = _build_bass(cfg, meta)
    in_maps = make_in_maps(xT, dinv_pp, idx_full,
                           W0, b0, W1, b1, W2, b2, Wc, bc)

    global LAST_NC, LAST_RESULTS
    LAST_NC = nc
    res = run_bass_kernel_spmd(nc, in_maps, core_ids=list(range(N_CORES)))
    LAST_RESULTS = res
    out = np.empty((cfg.n_nodes, N_CLS), dtype=np.float32)
    core_of = meta["core_of"]
    local_of = meta["local_of"]
    for c in range(N_CORES):
        sel = core_of == c
        out[sel] = res.results[c]["out"][local_of[sel]]
    return out
